# revision 3
# baseline (speedup 1.0000x reference)
"""Trainium2 Bass kernel for nn_ContrastiveLoss (SCAN text-to-image loss).

Full inputs in, full (scalar) output out. Internally: captions are sharded
across 8 NeuronCores (16 captions each, images replicated), each core
computes its scores[:, c_slice] block, an AllGather assembles the full
[128, 128] score matrix, and every core computes the diagonal-margin loss
redundantly; core 0's value is returned.

Math notes (exact reductions of the reference):
  - softmax over regions needs no normalizer: with E = exp(9 * a_norm),
    cos = (sum_r E*A) / (||cap|| * sqrt(E^T G E)) since the softmax
    normalizer Z cancels between numerator and denominator.
  - wei-norm uses the per-image Gram matrix G_i = X_i X_i^T, so the
    [C,I,W,D] weiContext tensor is never materialized.
  - word masking is folded into the caption operand (masked words become
    zero columns of the attention matrix).
  - 1/sqrt(x) is computed as exp(-0.5*ln(x)) so every scalar-engine
    function lives in one activation table (no 1.3us table reloads).
"""

import numpy as np

# Problem geometry (hardcoded per contract).
I, R, D, W = 128, 36, 512, 24
NCORES = 8
CS = I // NCORES          # captions per core = 16
GI = 3                    # images per PE group (3*36 = 108 <= 128 partitions)
GR = GI * R               # 108
NG = (I + GI - 1) // GI   # 43 groups
IRP = NG * GR             # 4644 padded image-region columns
CW = CS * W               # 384 caption-word columns per core
NK = D // 128             # 4 contraction chunks

_CACHE: dict = {}


def _build_program():
    import concourse.bacc as bacc
    import concourse.mybir as mybir
    import concourse.tile as tile

    f32 = mybir.dt.float32
    f32r = mybir.dt.float32r
    Act = mybir.ActivationFunctionType
    Alu = mybir.AluOpType
    X = mybir.AxisListType.X

    nc = bacc.Bacc("TRN2", target_bir_lowering=False, debug=False,
                   num_devices=NCORES)

    imT_d = nc.dram_tensor("imT", [NK, 128, IRP], f32, kind="ExternalInput")
    capT_d = nc.dram_tensor("capT", [NK, 128, CW], f32, kind="ExternalInput")
    wmask_d = nc.dram_tensor("wmask", [128, CW], f32, kind="ExternalInput")
    inds_d = nc.dram_tensor("inds", [GR, NG * 128], f32, kind="ExternalInput")
    eye_d = nc.dram_tensor("eye", [128, 128], f32, kind="ExternalInput")
    noteye_d = nc.dram_tensor("noteye", [128, 128], f32, kind="ExternalInput")
    ones_d = nc.dram_tensor("ones", [128, 128], f32, kind="ExternalInput")
    bmask_d = nc.dram_tensor("bmask", [GR, GR], f32, kind="ExternalInput")
    loss_d = nc.dram_tensor("loss", [1, 1], f32, kind="ExternalOutput")

    with tile.TileContext(nc) as tc:
        with (
            tc.tile_pool(name="const", bufs=1) as cp,
            tc.tile_pool(name="work", bufs=2) as wp,
            tc.tile_pool(name="imtp", bufs=3) as ip,
            tc.tile_pool(name="small", bufs=2) as sp,
            tc.tile_pool(name="dram", bufs=1, space="DRAM") as dp,
            tc.tile_pool(name="ps_acc", bufs=1, space="PSUM") as pa,
        ):
            # ---- constants -------------------------------------------------
            capT = [cp.tile([128, CW], f32, tag=f"capT{k}", name=f"capT{k}")
                    for k in range(NK)]
            capTm = [cp.tile([128, CW], f32, tag=f"capTm{k}", name=f"capTm{k}")
                     for k in range(NK)]
            wmask = cp.tile([128, CW], f32, tag="wmask")
            inds = cp.tile([GR, NG * 128], f32, tag="inds")
            eye = cp.tile([128, 128], f32, tag="eye")
            noteye = cp.tile([128, 128], f32, tag="noteye")
            ones = cp.tile([128, 128], f32, tag="ones")
            bmask = cp.tile([GR, GR], f32, tag="bmask")
            cn2 = cp.tile([128, CW], f32, tag="cn2")
            scores = cp.tile([128, CS], f32, tag="scores")
            scoresf = cp.tile([128, 128], f32, tag="scoresf")

            for k in range(NK):
                nc.sync.dma_start(out=capT[k][:], in_=capT_d[k])
            nc.sync.dma_start(out=wmask[:], in_=wmask_d[:])
            nc.sync.dma_start(out=inds[:].bitcast(f32r), in_=inds_d[:].bitcast(f32r))
            nc.sync.dma_start(out=eye[:].bitcast(f32r), in_=eye_d[:].bitcast(f32r))
            nc.sync.dma_start(out=noteye[:], in_=noteye_d[:])
            nc.sync.dma_start(out=ones[:].bitcast(f32r), in_=ones_d[:].bitcast(f32r))
            nc.sync.dma_start(out=bmask[:], in_=bmask_d[:])

            # ---- init: caption norms squared, replicated over partitions ---
            with tc.tile_pool(name="ps_init", bufs=1, space="PSUM") as pi:
                ps_cn2 = pi.tile([128, CW], f32, tag="cn2")
                for k in range(NK):
                    sq = wp.tile([128, CW], f32, tag="csq")
                    nc.gpsimd.tensor_tensor(
                        sq[:].bitcast(f32r), capT[k][:], capT[k][:], Alu.mult
                    )
                    nc.tensor.matmul(
                        ps_cn2[:], ones[:].bitcast(f32r), sq[:].bitcast(f32r),
                        start=(k == 0), stop=(k == NK - 1),
                    )
                nc.scalar.copy(cn2[:], ps_cn2[:])

            # masked captions feed the attention matmul
            for k in range(NK):
                nc.vector.tensor_tensor(
                    capTm[k][:].bitcast(f32r), capT[k][:], wmask[:], Alu.mult
                )

            # ---- accumulators for numE / q over all groups -----------------
            ps_ne = pa.tile([128, CW], f32, tag="ne")
            ps_q = pa.tile([128, CW], f32, tag="q")

            with (
                tc.tile_pool(name="ps_pair", bufs=2, space="PSUM") as ppair,
                tc.tile_pool(name="ps_gram", bufs=2, space="PSUM") as pgram,
            ):
                for g in range(NG):
                    imt = ip.tile([128, NK * GR], f32, tag="imt")
                    for k in range(NK):
                        nc.sync.dma_start(
                            out=imt[:, k * GR:(k + 1) * GR].bitcast(f32r),
                            in_=imT_d[k, :, g * GR:(g + 1) * GR].bitcast(f32r),
                        )
                    ps_pair = ppair.tile([GR, 1024], f32, tag="pair")
                    ps_gram = pgram.tile([GR, GR], f32, tag="gram")
                    for k in range(NK):
                        sl = imt[:, k * GR:(k + 1) * GR].bitcast(f32r)
                        nc.tensor.matmul(ps_gram[:], sl, sl,
                                         start=(k == 0), stop=(k == NK - 1))
                        nc.tensor.matmul(ps_pair[:, 0:CW], sl,
                                         capTm[k][:].bitcast(f32r),
                                         start=(k == 0), stop=(k == NK - 1))
                    # zero the cross-image blocks of the 3-image Gram
                    g_sb = sp.tile([GR, GR], f32, tag="gsb")
                    nc.vector.tensor_tensor(
                        g_sb[:].bitcast(f32r), ps_gram[:], bmask[:], Alu.mult
                    )
                    # B = leaky_relu(A) (masked words are zero cols already)
                    B = wp.tile([GR, CW], f32, tag="B")
                    nc.scalar.activation(B[:], ps_pair[:, 0:CW], Act.Prelu,
                                         alpha=0.1)
                    # n2 = sum_w B^2 per (row, caption)
                    B2 = wp.tile([GR, CW], f32, tag="B2")
                    nc.gpsimd.tensor_tensor(B2[:], B[:], B[:], Alu.mult)
                    n2 = sp.tile([GR, CS], f32, tag="n2")
                    nc.vector.reduce_sum(
                        n2[:], B2[:].rearrange("p (c w) -> p c w", w=W), axis=X
                    )
                    # rinv = n2^(-1/2) via exp(-0.5*ln)
                    lnn = sp.tile([GR, CS], f32, tag="lnn")
                    nc.scalar.activation(lnn[:], n2[:], Act.Ln)
                    rinv = sp.tile([GR, CS], f32, tag="rinv")
                    nc.scalar.activation(rinv[:], lnn[:], Act.Exp, scale=-0.5)
                    # Bn = B * rinv (broadcast over words)
                    Bn = wp.tile([GR, CW], f32, tag="Bn")
                    nc.gpsimd.tensor_tensor(
                        Bn[:].rearrange("p (c w) -> p c w", w=W),
                        B[:].rearrange("p (c w) -> p c w", w=W),
                        rinv[:].rearrange("p (c u) -> p c u", u=1)
                        .broadcast_to((GR, CS, W)),
                        Alu.mult,
                    )
                    # E = exp(9 * Bn)
                    E = wp.tile([GR, CW], f32, tag="E")
                    nc.scalar.activation(E[:].bitcast(f32r), Bn[:], Act.Exp,
                                         scale=9.0)
                    # GE = blockdiag(G) @ E  -> second psum bank
                    nc.tensor.matmul(ps_pair[:, 512:512 + CW],
                                     g_sb[:].bitcast(f32r),
                                     E[:].bitcast(f32r), start=True, stop=True)
                    # pair = [E*A | E*GE] in one DVE pass
                    pair = wp.tile([GR, 2 * CW], f32, tag="pair")
                    nc.vector.tensor_tensor(
                        pair[:].bitcast(f32r).rearrange("p (u f) -> p u f", u=2),
                        E[:].rearrange("p (u f) -> p u f", u=1)
                        .broadcast_to((GR, 2, CW)),
                        ps_pair[:].rearrange("p (u f) -> p u f", u=2)[:, :, 0:CW],
                        Alu.mult,
                    )
                    # block-sum over regions into the stacked accumulators
                    ind = inds[:, g * 128:(g + 1) * 128].bitcast(f32r)
                    nc.tensor.matmul(ps_ne[:], ind, pair[:, 0:CW].bitcast(f32r),
                                     start=(g == 0), stop=(g == NG - 1))
                    nc.tensor.matmul(ps_q[:], ind, pair[:, CW:2 * CW].bitcast(f32r),
                                     start=(g == 0), stop=(g == NG - 1))

            # ---- per-(image, word) epilogue: cos -> logsumexp --------------
            with tc.tile_pool(name="fin", bufs=1) as fp_, \
                 tc.tile_pool(name="ps_fin", bufs=1, space="PSUM") as pf:
                qc = fp_.tile([128, CW], f32, tag="qc")
                nc.vector.tensor_tensor(qc[:], ps_q[:], cn2[:], Alu.mult)
                lq = fp_.tile([128, CW], f32, tag="lq")
                nc.scalar.activation(lq[:], qc[:], Act.Ln)
                rsq = fp_.tile([128, CW], f32, tag="rsq")
                nc.scalar.activation(rsq[:], lq[:], Act.Exp, scale=-0.5)
                cosm = fp_.tile([128, CW], f32, tag="cosm")
                nc.vector.tensor_tensor(cosm[:], ps_ne[:], rsq[:], Alu.mult)
                ex = fp_.tile([128, CW], f32, tag="ex")
                nc.scalar.activation(ex[:], cosm[:], Act.Exp, scale=6.0)
                exm = fp_.tile([128, CW], f32, tag="exm")
                nc.vector.tensor_tensor(exm[:], ex[:], wmask[:], Alu.mult)
                rs = fp_.tile([128, CS], f32, tag="rs")
                nc.vector.reduce_sum(
                    rs[:], exm[:].rearrange("p (c w) -> p c w", w=W), axis=X
                )
                # scores (x6): L = ln(sum) = 6 * row_sim
                nc.scalar.activation(scores[:], rs[:], Act.Ln)

                # ---- all-gather the [128, 16] slices -----------------------
                sl_dram = dp.tile([128, CS], f32, name="sl_dram")
                ag_dram = dp.tile([NCORES, 128, CS], f32, name="ag_dram")
                nc.sync.dma_start(out=sl_dram[:], in_=scores[:])
                nc.gpsimd.collective_compute(
                    "AllGather", Alu.bypass,
                    replica_groups=[list(range(NCORES))],
                    ins=[sl_dram.opt()], outs=[ag_dram.opt()],
                )
                nc.sync.dma_start(
                    out=scoresf[:].rearrange("i (r j) -> i r j", r=NCORES),
                    in_=ag_dram[:].rearrange("r i j -> i r j"),
                )

                # ---- diagonal-margin loss on the full 6*scores matrix ------
                de = fp_.tile([128, 128], f32, tag="de")
                nc.vector.tensor_tensor(de[:], scoresf[:], eye[:], Alu.mult)
                diag = fp_.tile([128, 1], f32, tag="diag")
                nc.vector.reduce_sum(diag[:], de[:], axis=X)
                dm = fp_.tile([128, 1], f32, tag="dm")
                nc.vector.tensor_scalar(dm[:], diag[:], 1.2, None, Alu.subtract)
                m1 = fp_.tile([128, 128], f32, tag="m1")
                nc.vector.tensor_scalar(m1[:], scoresf[:], dm[:], 0.0,
                                        Alu.subtract, Alu.max)
                m1e = fp_.tile([128, 128], f32, tag="m1e")
                nc.vector.tensor_tensor(m1e[:], m1[:], noteye[:], Alu.mult)
                cs_ = fp_.tile([128, 1], f32, tag="cs")
                nc.vector.reduce_max(cs_[:], m1e[:], axis=X)

                ps_t = pf.tile([128, 128], f32, tag="t")
                nc.tensor.transpose(ps_t[:], scoresf[:], eye[:])
                m2 = fp_.tile([128, 128], f32, tag="m2")
                nc.vector.tensor_scalar(m2[:], ps_t[:], dm[:], 0.0,
                                        Alu.subtract, Alu.max)
                m2e = fp_.tile([128, 128], f32, tag="m2e")
                nc.vector.tensor_tensor(m2e[:], m2[:], noteye[:], Alu.mult)
                ci = fp_.tile([128, 1], f32, tag="ci")
                nc.vector.reduce_max(ci[:], m2e[:], axis=X)

                tt = fp_.tile([128, 1], f32, tag="tt")
                nc.vector.tensor_tensor(tt[:], cs_[:], ci[:], Alu.add)
                ps_l = pf.tile([1, 1], f32, tag="l")
                nc.tensor.matmul(ps_l[:], tt[:], ones[:, 0:1],
                                 start=True, stop=True)
                lsb = fp_.tile([1, 1], f32, tag="lsb")
                nc.scalar.mul(lsb[:], ps_l[:], 1.0 / 6.0)
                nc.sync.dma_start(out=loss_d[:], in_=lsb[:])

    nc.compile()
    return nc


def _prep_in_maps(images, captions, cap_lens):
    images = np.ascontiguousarray(images, dtype=np.float32)
    captions = np.ascontiguousarray(captions, dtype=np.float32)
    cap_lens = np.asarray(cap_lens, dtype=np.int32)

    imt = images.transpose(2, 0, 1).reshape(D, I * R)
    # pad the ragged last group with a dummy (real-valued) region block
    imt_p = np.concatenate([imt, imt[:, : IRP - I * R]], axis=1)
    imt_p = np.ascontiguousarray(imt_p).reshape(NK, 128, IRP)

    inds = np.zeros((GR, NG * 128), dtype=np.float32)
    for g in range(NG):
        for k in range(GR):
            m = GI * g + k // R
            if m < I:
                inds[k, g * 128 + m] = 1.0

    eye = np.eye(128, dtype=np.float32)
    noteye = (1.0 - eye).astype(np.float32)
    ones = np.ones((128, 128), dtype=np.float32)
    bmask = np.zeros((GR, GR), dtype=np.float32)
    for b in range(GI):
        bmask[b * R:(b + 1) * R, b * R:(b + 1) * R] = 1.0

    wvalid = (np.arange(W)[None, :] < cap_lens[:, None]).astype(np.float32)

    in_maps = []
    for r in range(NCORES):
        cap = captions[r * CS:(r + 1) * CS]                     # [16, 24, 512]
        capT = np.ascontiguousarray(
            cap.transpose(2, 0, 1).reshape(D, CW)
        ).reshape(NK, 128, CW)
        wm = np.ascontiguousarray(
            np.broadcast_to(
                wvalid[r * CS:(r + 1) * CS].reshape(1, CW), (128, CW)
            )
        ).astype(np.float32)
        in_maps.append({
            "imT": imt_p,
            "capT": capT,
            "wmask": wm,
            "inds": inds,
            "eye": eye,
            "noteye": noteye,
            "ones": ones,
            "bmask": bmask,
        })
    return in_maps


def _get_nc():
    if "nc" not in _CACHE:
        _CACHE["nc"] = _build_program()
    return _CACHE["nc"]


def kernel(images, captions, cap_lens):
    from concourse.bass_utils import run_bass_kernel_spmd

    nc = _get_nc()
    in_maps = _prep_in_maps(images, captions, cap_lens)
    res = run_bass_kernel_spmd(nc, in_maps, core_ids=list(range(NCORES)))
    out = res.results[0]["loss"]
    return np.float32(np.asarray(out).reshape(()))
